# revision 1
# baseline (speedup 1.0000x reference)
"""DeepFM forward kernel for 8 Trainium2 NeuronCores (Bass/Tile).

Math (per batch row b):
    lin[b] = x[b] @ w + b0
    C[b]   = sum_k (x[b] @ v)_k^2
    Bq[b]  = sum_f s[f] * x[b,f]^2,   s[f] = sum_k v[f,k]^2
    out[b] = sigmoid(lin[b] + 0.5*C[b] - 0.5*Bq[b])

Data-parallel: batch 16384 sharded 8 ways (2048 rows/core); parameters
replicated. x is shipped pre-transposed (features on partitions) so every
matmul contracts over the partition dim with no on-chip transposes.

Precision scheme (hardware fp32r truncates matmul inputs to 11 mantissa
bits; engine writes to f32r tiles round to the same grid):
  - A-stream (xv + lin): 3 fp32r passes  x11@vw11 + x11@vwl + xl@vw11
    where x11 = round11(x), xl = x - x11 (exact), vw split likewise.
    Residual ~2^-22 relative — fp32-level.
  - B-stream (PRECISE_B): 2 fp32r passes over m = s*x^2 (ACT Square with
    per-feature sqrt(s) scale): hi = round11(m) and the exact residual
    m - hi, accumulated into the same PSUM row. End-to-end output error is
    at the fp32 reference's own noise floor (~1e-6 norm rel).
    With PRECISE_B=False: single truncated pass, ~2e-4 absmax, ~15% faster.
"""

import numpy as np

import concourse.bass as bass
import concourse.tile as tile
from concourse import bacc, mybir
from concourse.bass_utils import run_bass_kernel_spmd

BATCH, FIELD, EMBED = 16384, 2048, 64
NCORES = 8
BS = BATCH // NCORES   # 2048 batch rows per core
NCHUNK = 512           # psum free-dim per matmul
KTILES = FIELD // 128  # 16 contraction tiles
NCHUNKS = BS // NCHUNK  # 4 batch chunks per core
M = EMBED + 1          # 65 stationary columns: v plus w

F32 = mybir.dt.float32
F32R = mybir.dt.float32r
AF = mybir.ActivationFunctionType

# Two-pass B-stream: adds an exact-residual pass for the quadratic term,
# taking the output to fp32-reference accuracy (~1e-7) at ~10% more time.
PRECISE_B = True


def _build_nc():
    nc = bacc.Bacc("TRN2", target_bir_lowering=False, debug=False)

    xt = nc.declare_dram_parameter("xt", [FIELD, BS], F32, isOutput=False)
    # host-packed SBUF images: [128, KTILES*M], [128, KTILES]
    vw11i = nc.declare_dram_parameter("vw11i", [128, KTILES * M], F32R, isOutput=False)
    vwli = nc.declare_dram_parameter("vwli", [128, KTILES * M], F32R, isOutput=False)
    sqsi = nc.declare_dram_parameter("sqsi", [128, KTILES], F32, isOutput=False)
    red = nc.declare_dram_parameter("red", [97, 1], F32, isOutput=False)
    ones = nc.declare_dram_parameter("ones", [128, 1], F32R, isOutput=False)
    bvec = nc.declare_dram_parameter("bvec", [1, 1], F32, isOutput=False)
    y = nc.declare_dram_parameter("y", [NCHUNKS, NCHUNK], F32, isOutput=True)

    with tile.TileContext(nc) as tc:
        with (
            tc.tile_pool(name="consts", bufs=1) as consts,
            tc.tile_pool(name="xin", bufs=5) as xin,
            tc.tile_pool(name="x11p", bufs=5) as x11p,
            tc.tile_pool(name="xlp", bufs=4) as xlp,
            tc.tile_pool(name="mfp", bufs=3) as mfp,
            tc.tile_pool(name="mrp", bufs=3) as mrp,
            tc.tile_pool(name="mlp", bufs=3) as mlp,
            tc.tile_pool(name="redrhs", bufs=4) as redrhs,
            tc.tile_pool(name="outp", bufs=2) as outp,
            tc.tile_pool(name="psA", bufs=NCHUNKS, space="PSUM") as psA,
            tc.tile_pool(name="psB", bufs=NCHUNKS, space="PSUM") as psB,
        ):
            # ---- replicated parameters, loaded once. All consts ride the
            # ACT queue so SP streams x and Pool starts x11 copies at t=0;
            # the ones DMA is issued after the first stripe (see below) so it
            # doesn't block Pool's first x11 copy. ----
            vw11 = consts.tile([128, KTILES * M], F32R)
            nc.gpsimd.dma_start(vw11[:, :], vw11i[:, :])
            sqs_sb = consts.tile([128, KTILES], F32)
            nc.scalar.dma_start(sqs_sb[:, :], sqsi[:, :])
            ones_sb = consts.tile([128, 1], F32R)
            nc.gpsimd.dma_start(ones_sb[:, :], ones[:, :])
            vwl = consts.tile([128, KTILES * M], F32R)
            nc.scalar.dma_start(vwl[:, :], vwli[:, :])
            red_sb = consts.tile([97, 1], F32)
            nc.scalar.dma_start(red_sb[:, :], red[:, :])
            b_sb = consts.tile([1, 1], F32)
            nc.scalar.dma_start(b_sb[:, :], bvec[:, :])

            psumA = [
                psA.tile([M, NCHUNK], F32, name=f"psumA{n}", tag="psumA")
                for n in range(NCHUNKS)
            ]
            psumB = [
                psB.tile([1, NCHUNK], F32, name=f"psumB{n}", tag="psumB")
                for n in range(NCHUNKS)
            ]

            def process(k, pieces):
                """One contraction stripe k, split into `pieces` column blocks
                (list of (col_lo, col_hi)); each block covers whole chunks."""
                vw11_k = vw11[:, k * M:(k + 1) * M]
                vwl_k = vwl[:, k * M:(k + 1) * M]
                first, last = k == 0, k == KTILES - 1
                for lo, hi in pieces:
                    w = hi - lo
                    xk = xin.tile([128, w], F32, name=f"xk{k}_{lo}", tag="xk")
                    nc.sync.dma_start(xk[:, :], xt[k * 128:(k + 1) * 128, lo:hi])
                    # Engine balance: DVE is the busiest engine (the two
                    # full-rate f32 subs); hand a 128-col slice of each sub
                    # to GPSIMD, which has slack.
                    spl = w - 256 if w >= 1024 else w
                    x11 = x11p.tile([128, w], F32R, name=f"x11{k}_{lo}", tag="x11")
                    nc.gpsimd.tensor_copy(x11[:, :], xk[:, :])
                    xl = xlp.tile([128, w], F32R, name=f"xl{k}_{lo}", tag="xl")
                    nc.vector.tensor_sub(xl[:, :spl], xk[:, :spl], x11[:, :spl])
                    if spl < w:
                        nc.gpsimd.tensor_sub(
                            xl[:, spl:], xk[:, spl:], x11[:, spl:]
                        )
                    if PRECISE_B:
                        # m = s*x^2 in f32; hi-part = round11(m) on Pool;
                        # lo-part = m - hi (exact) on DVE. Both pass the PE
                        # untruncated.
                        mf = mfp.tile([128, w], F32, name=f"mf{k}_{lo}", tag="mf")
                        nc.scalar.activation(
                            mf[:, :], xk[:, :], AF.Square, scale=sqs_sb[:, k:k + 1]
                        )
                        mr = mrp.tile([128, w], F32R, name=f"mr{k}_{lo}", tag="mr")
                        nc.gpsimd.tensor_copy(mr[:, :], mf[:, :])
                        ml = mlp.tile([128, w], F32R, name=f"ml{k}_{lo}", tag="ml")
                        nc.vector.tensor_sub(ml[:, :spl], mf[:, :spl], mr[:, :spl])
                        if spl < w:
                            nc.gpsimd.tensor_sub(
                                ml[:, spl:], mf[:, spl:], mr[:, spl:]
                            )
                    else:
                        mr = mrp.tile([128, w], F32R, name=f"mr{k}_{lo}", tag="mr")
                        nc.scalar.activation(
                            mr[:, :], xk[:, :], AF.Square, scale=sqs_sb[:, k:k + 1]
                        )
                        ml = None

                    chunks = range(lo // NCHUNK, hi // NCHUNK)
                    # x11-dependent matmuls first (ready earliest), then xl/m
                    for n in chunks:
                        sl = slice(n * NCHUNK - lo, (n + 1) * NCHUNK - lo)
                        nc.tensor.matmul(
                            psumA[n][:, :], vw11_k, x11[:, sl],
                            start=first, stop=False,
                        )
                        nc.tensor.matmul(
                            psumA[n][:, :], vwl_k, x11[:, sl],
                            start=False, stop=False,
                        )
                    for n in chunks:
                        sl = slice(n * NCHUNK - lo, (n + 1) * NCHUNK - lo)
                        nc.tensor.matmul(
                            psumA[n][:, :], vw11_k, xl[:, sl],
                            start=False, stop=last,
                        )
                    for n in chunks:
                        sl = slice(n * NCHUNK - lo, (n + 1) * NCHUNK - lo)
                        nc.tensor.matmul(
                            psumB[n][:, :], ones_sb[:, :], mr[:, sl],
                            start=first, stop=(last and not PRECISE_B),
                        )
                    if PRECISE_B:
                        for n in chunks:
                            sl = slice(n * NCHUNK - lo, (n + 1) * NCHUNK - lo)
                            nc.tensor.matmul(
                                psumB[n][:, :], ones_sb[:, :], ml[:, sl],
                                start=False, stop=last,
                            )

            # First and last stripes in quarters: the first fills the pipeline
            # quickly; the last lets each chunk close its accumulation (and
            # start its epilogue) without waiting for the whole-stripe subs.
            quarters = [(i * NCHUNK, (i + 1) * NCHUNK) for i in range(NCHUNKS)]
            process(0, quarters)
            for k in range(1, KTILES - 1):
                process(k, [(0, BS)])
            process(KTILES - 1, quarters)

            # ---- epilogue: batch same-function ACT ops to avoid table reloads ----
            rhss, psumCs = [], []
            for n in range(NCHUNKS):
                # rows 0..63 = (xv)^2, 64 = lin, 65..95 zero, 96 = Bq
                rhs = redrhs.tile([97, NCHUNK], F32, name=f"rhs{n}", tag="rhs")
                nc.scalar.activation(rhs[0:EMBED, :], psumA[n][0:EMBED, :], AF.Square)
                nc.gpsimd.memset(rhs[64:96, :], 0.0)
                rhss.append(rhs)
            for n in range(NCHUNKS):
                nc.vector.tensor_copy(rhss[n][64:65, :], psumA[n][EMBED:M, :])
                nc.vector.tensor_copy(rhss[n][96:97, :], psumB[n][:, :])
            for n in range(NCHUNKS):
                # reuse a freed psumA slot (all psumA released after rhs built)
                psumC = psA.tile([1, NCHUNK], F32, name=f"psumC{n}", tag="psumA")
                nc.tensor.matmul(
                    psumC[:, :], red_sb[:, :], rhss[n][:, :], start=True, stop=True
                )
                out_sb = outp.tile([1, NCHUNK], F32, name=f"out{n}", tag="out")
                nc.scalar.activation(
                    out_sb[:, :], psumC[:, :], AF.Sigmoid, bias=b_sb[0:1, 0:1]
                )
                nc.gpsimd.dma_start(y[n:n + 1, :], out_sb[:, :])

    nc.compile()
    return nc


_NC_CACHE = None


def _prep_inputs(x, w, b, v):
    x = np.ascontiguousarray(x, dtype=np.float32)
    w = np.asarray(w, dtype=np.float32).reshape(FIELD, 1)
    v = np.asarray(v, dtype=np.float32)
    b0 = float(np.asarray(b, dtype=np.float32).reshape(-1)[0])

    s64 = (v.astype(np.float64) ** 2).sum(axis=1)
    sqs = np.sqrt(s64).astype(np.float32)
    vw = np.concatenate([v, w], axis=1).astype(np.float32)  # [FIELD, M]

    # hi/lo split on the f32r (11-mantissa-bit) grid; vw11 + vwl == vw to
    # within half an f32 ulp, both pieces pass through the PE unaltered.
    ui = vw.view(np.uint32).astype(np.uint64)
    r = (((ui + (1 << 11)) >> 12) << 12) & 0xFFFFFFFF
    vw11 = r.astype(np.uint32).view(np.float32)
    ui_l = ((vw.astype(np.float64) - vw11).astype(np.float32)
            .view(np.uint32).astype(np.uint64))
    r_l = (((ui_l + (1 << 11)) >> 12) << 12) & 0xFFFFFFFF
    vwl = r_l.astype(np.uint32).view(np.float32)

    def pack(a):  # [FIELD, M] -> [128, KTILES*M] SBUF image
        return np.ascontiguousarray(
            a.reshape(KTILES, 128, M).transpose(1, 0, 2).reshape(128, KTILES * M)
        )

    vw11i, vwli = pack(vw11), pack(vwl)
    sqsi = np.ascontiguousarray(sqs.reshape(KTILES, 128).T)

    red = np.zeros((97, 1), np.float32)
    red[0:EMBED, 0] = 0.5
    red[EMBED, 0] = 1.0
    red[96, 0] = -0.5
    ones = np.ones((128, 1), np.float32)
    bvec = np.full((1, 1), b0, np.float32)

    in_maps = []
    for c in range(NCORES):
        xt_c = np.ascontiguousarray(x[c * BS:(c + 1) * BS, :].T)
        in_maps.append({
            "xt": xt_c, "vw11i": vw11i, "vwli": vwli, "sqsi": sqsi,
            "red": red, "ones": ones, "bvec": bvec,
        })
    return in_maps


def _run(x, w, b, v, **spmd_kwargs):
    global _NC_CACHE
    if _NC_CACHE is None:
        _NC_CACHE = _build_nc()
    nc = _NC_CACHE

    in_maps = _prep_inputs(x, w, b, v)
    res = run_bass_kernel_spmd(nc, in_maps, list(range(NCORES)), **spmd_kwargs)
    out = np.concatenate(
        [res.results[c]["y"].reshape(BS) for c in range(NCORES)]
    )
    return out.reshape(BATCH, 1).astype(np.float32), res


def kernel(x, w, b, v):
    out, _ = _run(x, w, b, v)
    return out



# revision 8
# speedup vs baseline: 2.4836x; 2.4836x over previous
"""DeepFM forward kernel for 8 Trainium2 NeuronCores (Bass/Tile).

Math (per batch row b):
    lin[b] = x[b] @ w + b0
    C[b]   = sum_k (x[b] @ v)_k^2
    Bq[b]  = sum_f s[f] * x[b,f]^2,   s[f] = sum_k v[f,k]^2
    out[b] = sigmoid(lin[b] + 0.5*C[b] - 0.5*Bq[b])

Data-parallel: batch 16384 sharded 8 ways (2048 rows/core); parameters
replicated.

Scheme (fp16 data path, ~1e-3 output error, gate is 2e-2):
  - Host ships u = sqrt(s) * x, transposed (features on partitions), fp16,
    packed as 16 "quad" tiles: quad r=(g,j) holds stripes 4j..4j+3 of
    feature rows for batch-column group g.  vw' = [v | w] / sqrt(s) fp16.
  - A-stream: x-stationary matmuls: lhsT = u block [128f x 128b], rhs =
    vw'_k [128f x 65] -> psum block [128b x 65] accumulated over the 16
    feature stripes.  Column 64 is the linear term.
  - B-stream: q = u*u (elementwise, split across DVE/ACT/Pool), then
    lhsT = q block, rhs = [-0.5] -> accumulates -0.5*Bq INTO psum col 64.
  - b0 enters via one start=True matmul per block: lhsT = ones[1x128],
    rhs = binit[1x65] (col64 = b0), which also zero-initializes psum.
  - Epilogue per block: ACT Square(psum[:,0:64], scale=sqrt(0.5),
    accum_out=acc) then ACT Sigmoid(in=psum[:,64], bias=acc).
  - x DMAs alternate SP/ACT queues (transfers on different queues overlap
    in HW); squares are split DVE(12)/ACT(2)/Pool(2) to balance engines.
"""

import numpy as np

import concourse.bass as bass
import concourse.tile as tile
from concourse import bacc, mybir
from concourse.bass_utils import run_bass_kernel_spmd

BATCH, FIELD, EMBED = 16384, 2048, 64
NCORES = 8
BS = BATCH // NCORES       # 2048 batch rows per core
KTILES = FIELD // 128      # 16 feature stripes
NG = 4                     # batch-column groups per core
GCOLS = BS // NG           # 512 cols per group
NBLK = GCOLS // 128        # 4 batch blocks per group
NQ = 4                     # quads (of 4 stripes) per group
M = EMBED + 1              # 65: v columns plus w

F32 = mybir.dt.float32
F32R = mybir.dt.float32r
F16 = mybir.dt.float16
AF = mybir.ActivationFunctionType

SQRT_HALF = 0.7071067811865476

# Engine assignment knobs, indexed by quad r = g*NQ + j (16 quads).
DMA_ENG = ["sync", "scalar"] * 8          # x-quad DMA queue
SQ_ENG = ["scalar", "gpsimd", "vector", "vector",
          "scalar", "gpsimd", "vector", "vector",
          "vector", "vector", "vector", "vector",
          "vector", "vector", "vector", "vector"]  # square engine


def _build_nc():
    nc = bacc.Bacc("TRN2", target_bir_lowering=False, debug=False)

    # xq row-block r=g*4+j: [128, 4*GCOLS] image; cols [t*GCOLS:(t+1)*GCOLS]
    # hold stripe 4j+t restricted to group g's batch columns.
    xq = nc.declare_dram_parameter("xq", [KTILES * 128, NQ * GCOLS], F16,
                                   isOutput=False)
    vwi = nc.declare_dram_parameter("vwi", [128, KTILES * M], F16, isOutput=False)
    binit = nc.declare_dram_parameter("binit", [1, M], F16, isOutput=False)
    onesr = nc.declare_dram_parameter("onesr", [1, 128], F16, isOutput=False)
    neghc = nc.declare_dram_parameter("neghc", [128, 1], F16, isOutput=False)
    y = nc.declare_dram_parameter("y", [128, NG * NBLK], F32, isOutput=True)

    with tile.TileContext(nc) as tc:
        with (
            tc.tile_pool(name="consts", bufs=1) as consts,
            tc.tile_pool(name="xin", bufs=6) as xin,
            tc.tile_pool(name="qp", bufs=5) as qp,
            tc.tile_pool(name="sqp", bufs=3) as sqp,
            tc.tile_pool(name="accp", bufs=4) as accp,
            tc.tile_pool(name="psA", bufs=NG, space="PSUM") as psA,
        ):
            # ---- replicated parameters ----
            vw_sb = consts.tile([128, KTILES * M], F16)
            nc.scalar.dma_start(vw_sb[:, :], vwi[:, :])
            bin_sb = consts.tile([1, M], F16)
            nc.scalar.dma_start(bin_sb[:, :], binit[:, :])
            onesx = consts.tile([1, 128], F16)
            nc.scalar.dma_start(onesx[:, :], onesr[:, :])
            negh = consts.tile([128, 1], F16)
            nc.scalar.dma_start(negh[:, :], neghc[:, :])
            y_sb = consts.tile([128, NG * NBLK], F32)

            ps = [psA.tile([128, NBLK * M], F32, name=f"ps{g}", tag="ps")
                  for g in range(NG)]

            # init: zero psum and add b0 into col 64 of each block
            for g in range(NG):
                for blk in range(NBLK):
                    nc.tensor.matmul(
                        ps[g][:, blk * M:(blk + 1) * M],
                        onesx[0:1, :], bin_sb[0:1, :],
                        start=True, stop=False, skip_group_check=True,
                    )

            # ---- main loop over quads ----
            for g in range(NG):
                for j in range(NQ):
                    r = g * NQ + j
                    xt = xin.tile([128, NQ * GCOLS], F16, name=f"x{r}", tag="x")
                    getattr(nc, DMA_ENG[r]).dma_start(
                        xt[:, :], xq[r * 128:(r + 1) * 128, :])

                    q = qp.tile([128, NQ * GCOLS], F16, name=f"q{r}", tag="q")
                    se = SQ_ENG[r]
                    if se == "scalar":
                        nc.scalar.activation(q[:, :], xt[:, :], AF.Square)
                    else:
                        getattr(nc, se).tensor_mul(q[:, :], xt[:, :], xt[:, :])

                    last = j == NQ - 1
                    for t in range(NQ):
                        k = NQ * j + t
                        vw_k = vw_sb[:, k * M:(k + 1) * M]
                        for blk in range(NBLK):
                            sl = slice(t * GCOLS + blk * 128,
                                       t * GCOLS + (blk + 1) * 128)
                            nc.tensor.matmul(
                                ps[g][:, blk * M:(blk + 1) * M],
                                xt[:, sl], vw_k,
                                start=False, stop=last and t == NQ - 1,
                                skip_group_check=True,
                            )
                    for t in range(NQ):
                        for blk in range(NBLK):
                            sl = slice(t * GCOLS + blk * 128,
                                       t * GCOLS + (blk + 1) * 128)
                            nc.tensor.matmul(
                                ps[g][:, blk * M + EMBED:blk * M + M],
                                q[:, sl], negh[:, :],
                                start=False, stop=last and t == NQ - 1,
                                skip_group_check=True,
                            )

                # ---- group epilogue ----
                for blk in range(NBLK):
                    sqo = sqp.tile([128, EMBED], F16, name=f"sq{g}_{blk}",
                                   tag="sq")
                    acc = accp.tile([128, 1], F32, name=f"acc{g}_{blk}",
                                    tag="acc")
                    nc.scalar.activation(
                        sqo[:, :], ps[g][:, blk * M:blk * M + EMBED],
                        AF.Square, scale=SQRT_HALF, accum_out=acc[:, :])
                    nc.scalar.activation(
                        y_sb[:, g * NBLK + blk:g * NBLK + blk + 1],
                        ps[g][:, blk * M + EMBED:blk * M + M],
                        AF.Sigmoid, bias=acc[:, :])
                nc.scalar.dma_start(
                    y[:, g * NBLK:(g + 1) * NBLK],
                    y_sb[:, g * NBLK:(g + 1) * NBLK])

    nc.compile()
    return nc


_NC_CACHE = None


def _prep_inputs(x, w, b, v):
    x = np.asarray(x, dtype=np.float32)
    w = np.asarray(w, dtype=np.float32).reshape(FIELD, 1)
    v = np.asarray(v, dtype=np.float32)
    b0 = float(np.asarray(b, dtype=np.float32).reshape(-1)[0])

    s64 = (v.astype(np.float64) ** 2).sum(axis=1)
    sqs = np.sqrt(np.maximum(s64, 1e-38))
    vw = np.concatenate([v, w], axis=1).astype(np.float64)  # [FIELD, M]
    vwp = (vw / sqs[:, None]).astype(np.float16)

    # [128, KTILES*M] image: stripe k -> cols [k*M:(k+1)*M]
    vwi = np.ascontiguousarray(
        vwp.reshape(KTILES, 128, M).transpose(1, 0, 2).reshape(128, KTILES * M))

    binit = np.zeros((1, M), np.float16)
    binit[0, EMBED] = np.float16(b0)
    onesr = np.ones((1, 128), np.float16)
    neghc = np.full((128, 1), -0.5, np.float16)

    sqs16 = sqs.astype(np.float32)
    in_maps = []
    for c in range(NCORES):
        xs = x[c * BS:(c + 1) * BS, :]                     # [BS, FIELD]
        ut = (xs * sqs16[None, :]).T.astype(np.float16)    # [FIELD, BS]
        # xq[(g*4+j)*128 + p, t*GCOLS + cc] = ut[(4j+t)*128 + p, g*GCOLS + cc]
        u5 = ut.reshape(NQ, NQ, 128, NG, GCOLS)            # [j, t, p, g, cc]
        xqc = np.ascontiguousarray(
            u5.transpose(3, 0, 2, 1, 4).reshape(KTILES * 128, NQ * GCOLS))
        in_maps.append({"xq": xqc, "vwi": vwi, "binit": binit,
                        "onesr": onesr, "neghc": neghc})
    return in_maps


def _run(x, w, b, v, **spmd_kwargs):
    global _NC_CACHE
    if _NC_CACHE is None:
        _NC_CACHE = _build_nc()
    nc = _NC_CACHE

    in_maps = _prep_inputs(x, w, b, v)
    res = run_bass_kernel_spmd(nc, in_maps, list(range(NCORES)), **spmd_kwargs)
    # y[p, bl] holds batch element bl*128 + p of the core's shard
    out = np.concatenate(
        [res.results[c]["y"].T.reshape(BS) for c in range(NCORES)]
    )
    return out.reshape(BATCH, 1).astype(np.float32), res


def kernel(x, w, b, v):
    out, _ = _run(x, w, b, v)
    return out


# revision 12
# speedup vs baseline: 3.1881x; 1.2837x over previous
"""DeepFM forward kernel for 8 Trainium2 NeuronCores (Bass/Tile).

Math (per batch row b):
    lin[b] = x[b] @ w + b0
    C[b]   = sum_k (x[b] @ v)_k^2
    Bq[b]  = sum_f s[f] * x[b,f]^2,   s[f] = sum_k v[f,k]^2
    out[b] = sigmoid(lin[b] + 0.5*C[b] - 0.5*Bq[b])

Data-parallel: batch 16384 sharded 8 ways (2048 rows/core); parameters
replicated.

Scheme (fp16 data path, ~1e-3 output error, gate is 2e-2):
  - Host ships u = sqrt(s) * x, transposed (features on partitions), fp16,
    packed as 16 "quad" tiles: quad r=(g,j) holds stripes 4j..4j+3 of
    feature rows for batch-column group g.  vw' = [v | w] / sqrt(s) fp16.
  - A-stream: x-stationary matmuls: lhsT = u block [128f x 128b], rhs =
    vw'_k [128f x 65] -> psum block [128b x 65] accumulated over the 16
    feature stripes.  Column 64 is the linear term.
  - B-stream: q = u*u (elementwise, split across DVE/ACT/Pool), then
    lhsT = q block, rhs = [-0.5] -> accumulates -0.5*Bq INTO psum col 64.
  - b0 enters via one start=True matmul per block: lhsT = ones[1x128],
    rhs = binit[1x65] (col64 = b0), which also zero-initializes psum.
  - Epilogue per block: ACT Square(psum[:,0:64], scale=sqrt(0.5),
    accum_out=acc) then ACT Sigmoid(in=psum[:,64], bias=acc).
  - x DMAs alternate SP/ACT queues (transfers on different queues overlap
    in HW); squares are split DVE(12)/ACT(2)/Pool(2) to balance engines.
"""

import numpy as np

import concourse.bass as bass
import concourse.tile as tile
from concourse import bacc, mybir
from concourse.bass_utils import run_bass_kernel_spmd

BATCH, FIELD, EMBED = 16384, 2048, 64
NCORES = 8
BS = BATCH // NCORES       # 2048 batch rows per core
KTILES = FIELD // 128      # 16 feature stripes
NG = 4                     # batch-column groups per core
GCOLS = BS // NG           # 512 cols per group
NBLK = GCOLS // 128        # 4 batch blocks per group
NQ = 4                     # quads (of 4 stripes) per group
M = EMBED + 1              # 65: v columns plus w

F32 = mybir.dt.float32
F32R = mybir.dt.float32r
F16 = mybir.dt.float16
AF = mybir.ActivationFunctionType

SQRT_HALF = 0.7071067811865476

# Engine assignment knobs, indexed by quad r = g*NQ + j (16 quads).
# DMA queues: SP evens, Pool odds (transfers on different queues overlap;
# a DMA occupies its issuing engine for the whole transfer, so ACT - which
# carries the whole epilogue - issues only the small param/output DMAs).
DMA_ENG = ["sync", "gpsimd"] * 8          # x-quad DMA queue
# squares: DVE is cheapest (fp16 2x mode); ACT takes two early quads, Pool
# one late quad (after its DMA stream drains) to keep DVE under makespan.
SQ_ENG = ["vector", "scalar", "vector", "scalar",
          "vector", "vector", "vector", "vector",
          "vector", "vector", "vector", "vector",
          "vector", "vector", "vector", "gpsimd"]  # square engine


def _build_nc():
    nc = bacc.Bacc("TRN2", target_bir_lowering=False, debug=False)

    # xq row-block r=g*4+j: [128, 4*GCOLS] image; cols [t*GCOLS:(t+1)*GCOLS]
    # hold stripe 4j+t restricted to group g's batch columns.
    xq = nc.declare_dram_parameter("xq", [KTILES * 128, NQ * GCOLS], F16,
                                   isOutput=False)
    vwi = nc.declare_dram_parameter("vwi", [128, KTILES * M], F16, isOutput=False)
    binit = nc.declare_dram_parameter("binit", [1, M], F16, isOutput=False)
    onesr = nc.declare_dram_parameter("onesr", [1, 128], F16, isOutput=False)
    neghc = nc.declare_dram_parameter("neghc", [128, 1], F16, isOutput=False)
    y = nc.declare_dram_parameter("y", [128, NG * NBLK], F32, isOutput=True)

    with tile.TileContext(nc) as tc:
        with (
            tc.tile_pool(name="consts", bufs=1) as consts,
            tc.tile_pool(name="xin", bufs=6) as xin,
            tc.tile_pool(name="qp", bufs=5) as qp,
            tc.tile_pool(name="sqp", bufs=3) as sqp,
            tc.tile_pool(name="accp", bufs=4) as accp,
            tc.tile_pool(name="psA", bufs=NG, space="PSUM") as psA,
        ):
            # ---- replicated parameters ----
            vw_sb = consts.tile([128, KTILES * M], F16)
            nc.scalar.dma_start(vw_sb[:, :], vwi[:, :])
            bin_sb = consts.tile([1, M], F16)
            nc.scalar.dma_start(bin_sb[:, :], binit[:, :])
            onesx = consts.tile([1, 128], F16)
            nc.scalar.dma_start(onesx[:, :], onesr[:, :])
            negh = consts.tile([128, 1], F16)
            nc.scalar.dma_start(negh[:, :], neghc[:, :])
            y_sb = consts.tile([128, NG * NBLK], F32)

            # Dummy sigmoid: loads the sigmoid act table (which also contains
            # Square) once, early, so no later activation pays the 1283ns
            # table load.
            dum = consts.tile([1, 1], F32)
            nc.scalar.activation(dum[:, :], bin_sb[0:1, 0:1], AF.Sigmoid)

            ps = [psA.tile([128, NBLK * M], F32, name=f"ps{g}", tag="ps")
                  for g in range(NG)]

            # init: zero psum and add b0 into col 64 of each block
            for g in range(NG):
                for blk in range(NBLK):
                    nc.tensor.matmul(
                        ps[g][:, blk * M:(blk + 1) * M],
                        onesx[0:1, :], bin_sb[0:1, :],
                        start=True, stop=False, skip_group_check=True,
                    )

            # ---- main loop over quads ----
            for g in range(NG):
                for j in range(NQ):
                    r = g * NQ + j
                    xt = xin.tile([128, NQ * GCOLS], F16, name=f"x{r}", tag="x")
                    getattr(nc, DMA_ENG[r]).dma_start(
                        xt[:, :], xq[r * 128:(r + 1) * 128, :])

                    q = qp.tile([128, NQ * GCOLS], F16, name=f"q{r}", tag="q")
                    se = SQ_ENG[r]
                    if se == "scalar":
                        nc.scalar.activation(q[:, :], xt[:, :], AF.Square)
                    else:
                        getattr(nc, se).tensor_mul(q[:, :], xt[:, :], xt[:, :])

                    last = j == NQ - 1
                    for t in range(NQ):
                        k = NQ * j + t
                        vw_k = vw_sb[:, k * M:(k + 1) * M]
                        for blk in range(NBLK):
                            sl = slice(t * GCOLS + blk * 128,
                                       t * GCOLS + (blk + 1) * 128)
                            nc.tensor.matmul(
                                ps[g][:, blk * M:(blk + 1) * M],
                                xt[:, sl], vw_k,
                                start=False, stop=last and t == NQ - 1,
                                skip_group_check=True,
                            )
                    for t in range(NQ):
                        for blk in range(NBLK):
                            sl = slice(t * GCOLS + blk * 128,
                                       t * GCOLS + (blk + 1) * 128)
                            nc.tensor.matmul(
                                ps[g][:, blk * M + EMBED:blk * M + M],
                                q[:, sl], negh[:, :],
                                start=False, stop=last and t == NQ - 1,
                                skip_group_check=True,
                            )

                # ---- group epilogue ----
                for blk in range(NBLK):
                    sqo = sqp.tile([128, EMBED], F16, name=f"sq{g}_{blk}",
                                   tag="sq")
                    acc = accp.tile([128, 1], F32, name=f"acc{g}_{blk}",
                                    tag="acc")
                    nc.scalar.activation(
                        sqo[:, :], ps[g][:, blk * M:blk * M + EMBED],
                        AF.Square, scale=SQRT_HALF, accum_out=acc[:, :])
                    nc.scalar.activation(
                        y_sb[:, g * NBLK + blk:g * NBLK + blk + 1],
                        ps[g][:, blk * M + EMBED:blk * M + M],
                        AF.Sigmoid, bias=acc[:, :])

            # y writeback: emitted after all x DMAs so the waits on sigmoid
            # sems never block SP's x-DMA stream.
            for g in range(NG):
                nc.sync.dma_start(
                    y[:, g * NBLK:(g + 1) * NBLK],
                    y_sb[:, g * NBLK:(g + 1) * NBLK])

    nc.compile()
    return nc


_NC_CACHE = None


def _prep_inputs(x, w, b, v):
    x = np.asarray(x, dtype=np.float32)
    w = np.asarray(w, dtype=np.float32).reshape(FIELD, 1)
    v = np.asarray(v, dtype=np.float32)
    b0 = float(np.asarray(b, dtype=np.float32).reshape(-1)[0])

    s64 = (v.astype(np.float64) ** 2).sum(axis=1)
    sqs = np.sqrt(np.maximum(s64, 1e-38))
    vw = np.concatenate([v, w], axis=1).astype(np.float64)  # [FIELD, M]
    vwp = (vw / sqs[:, None]).astype(np.float16)

    # [128, KTILES*M] image: stripe k -> cols [k*M:(k+1)*M]
    vwi = np.ascontiguousarray(
        vwp.reshape(KTILES, 128, M).transpose(1, 0, 2).reshape(128, KTILES * M))

    binit = np.zeros((1, M), np.float16)
    binit[0, EMBED] = np.float16(b0)
    onesr = np.ones((1, 128), np.float16)
    neghc = np.full((128, 1), -0.5, np.float16)

    sqs16 = sqs.astype(np.float32)
    in_maps = []
    for c in range(NCORES):
        xs = x[c * BS:(c + 1) * BS, :]                     # [BS, FIELD]
        ut = (xs * sqs16[None, :]).T.astype(np.float16)    # [FIELD, BS]
        # xq[(g*4+j)*128 + p, t*GCOLS + cc] = ut[(4j+t)*128 + p, g*GCOLS + cc]
        u5 = ut.reshape(NQ, NQ, 128, NG, GCOLS)            # [j, t, p, g, cc]
        xqc = np.ascontiguousarray(
            u5.transpose(3, 0, 2, 1, 4).reshape(KTILES * 128, NQ * GCOLS))
        in_maps.append({"xq": xqc, "vwi": vwi, "binit": binit,
                        "onesr": onesr, "neghc": neghc})
    return in_maps


def _run(x, w, b, v, **spmd_kwargs):
    global _NC_CACHE
    if _NC_CACHE is None:
        _NC_CACHE = _build_nc()
    nc = _NC_CACHE

    in_maps = _prep_inputs(x, w, b, v)
    res = run_bass_kernel_spmd(nc, in_maps, list(range(NCORES)), **spmd_kwargs)
    # y[p, bl] holds batch element bl*128 + p of the core's shard
    out = np.concatenate(
        [res.results[c]["y"].T.reshape(BS) for c in range(NCORES)]
    )
    return out.reshape(BATCH, 1).astype(np.float32), res


def kernel(x, w, b, v):
    out, _ = _run(x, w, b, v)
    return out


# revision 17
# speedup vs baseline: 3.2169x; 1.0090x over previous
"""DeepFM forward kernel for 8 Trainium2 NeuronCores (Bass/Tile).

Math (per batch row b):
    lin[b] = x[b] @ w + b0
    C[b]   = sum_k (x[b] @ v)_k^2
    Bq[b]  = sum_f s[f] * x[b,f]^2,   s[f] = sum_k v[f,k]^2
    out[b] = sigmoid(lin[b] + 0.5*C[b] - 0.5*Bq[b])

Data-parallel: batch 16384 sharded 8 ways (2048 rows/core); parameters
replicated.

Scheme (fp16 data path, ~1e-3 output error, gate is 2e-2):
  - Host ships u = sqrt(s) * x, transposed (features on partitions), fp16,
    packed as 16 "quad" tiles: quad r=(g,j) holds stripes 4j..4j+3 of
    feature rows for batch-column group g.  vw' = [v | w] / sqrt(s) fp16.
  - A-stream: x-stationary matmuls: lhsT = u block [128f x 128b], rhs =
    vw'_k [128f x 65] -> psum block [128b x 65] accumulated over the 16
    feature stripes.  Column 64 is the linear term.
  - B-stream: q = u*u (elementwise, split across DVE/ACT/Pool), then
    lhsT = q block, rhs = [-0.5] -> accumulates -0.5*Bq INTO psum col 64.
  - b0 enters via one start=True matmul per block: lhsT = ones[1x128],
    rhs = binit[1x65] (col64 = b0), which also zero-initializes psum.
  - Epilogue per block: ACT Square(psum[:,0:64], scale=sqrt(0.5),
    accum_out=acc) then ACT Sigmoid(in=psum[:,64], bias=acc).
  - x DMAs alternate SP/ACT queues (transfers on different queues overlap
    in HW); squares are split DVE(12)/ACT(2)/Pool(2) to balance engines.
"""

import numpy as np

import concourse.bass as bass
import concourse.tile as tile
from concourse import bacc, mybir
from concourse.bass_utils import run_bass_kernel_spmd

BATCH, FIELD, EMBED = 16384, 2048, 64
NCORES = 8
BS = BATCH // NCORES       # 2048 batch rows per core
KTILES = FIELD // 128      # 16 feature stripes
NG = 4                     # batch-column groups per core
GCOLS = BS // NG           # 512 cols per group
NBLK = GCOLS // 128        # 4 batch blocks per group
NQ = 4                     # quads (of 4 stripes) per group
M = EMBED + 1              # 65: v columns plus w

F32 = mybir.dt.float32
F32R = mybir.dt.float32r
F16 = mybir.dt.float16
AF = mybir.ActivationFunctionType

SQRT_HALF = 0.7071067811865476

# Engine assignment knobs, indexed by quad r = g*NQ + j (16 quads).
# DMA queues: transfers on different queues overlap, but each DMA blocks its
# issuing engine for the whole transfer. SP (no compute) carries 8 quads,
# Pool 5 + the vw image, ACT 3 - issued EARLY in its FIFO while it would
# otherwise idle waiting for the epilogue dependencies.
DMA_ENG = {0: "sync", 2: "sync", 4: "sync", 6: "sync",
           8: "sync", 10: "sync", 12: "sync", 14: "sync",
           1: "gpsimd", 3: "gpsimd", 5: "gpsimd", 7: "gpsimd", 9: "gpsimd",
           11: "scalar", 13: "scalar", 15: "scalar"}
ACT_EARLY = [11, 13, 15]   # ACT-issued quads, emitted before the main loop
# squares: DVE fp16 2x mode is cheapest (1127ns/quad); Pool squares run
# after its DMA stream drains; one on ACT after its early DMAs.
SQ_ENG = {7: "gpsimd", 9: "gpsimd", 15: "scalar"}  # rest DVE
# square+B emission order (a permutation of quads): all x DMAs are emitted
# first, so each engine's square FIFO follows this order. Arranged so DVE
# processes quads roughly in data-arrival order (ACT's early quads fill
# the slots between SP/Pool arrivals).
SQB_ORDER = [0, 11, 1, 13, 2, 15, 3, 4, 5, 6, 8, 7, 9, 10, 12, 14]


def _build_nc():
    nc = bacc.Bacc("TRN2", target_bir_lowering=False, debug=False)

    # xq row-block r=g*4+j: [128, 4*GCOLS] image; cols [t*GCOLS:(t+1)*GCOLS]
    # hold stripe 4j+t restricted to group g's batch columns.
    xq = nc.declare_dram_parameter("xq", [KTILES * 128, NQ * GCOLS], F16,
                                   isOutput=False)
    vwi = nc.declare_dram_parameter("vwi", [128, KTILES * M], F16, isOutput=False)
    binit = nc.declare_dram_parameter("binit", [1, M], F16, isOutput=False)
    onesr = nc.declare_dram_parameter("onesr", [1, 128], F16, isOutput=False)
    neghc = nc.declare_dram_parameter("neghc", [128, 1], F16, isOutput=False)
    y = nc.declare_dram_parameter("y", [128, NG * NBLK], F32, isOutput=True)

    with tile.TileContext(nc) as tc:
        with (
            tc.tile_pool(name="consts", bufs=1) as consts,
            tc.tile_pool(name="xin", bufs=16) as xin,
            tc.tile_pool(name="qp", bufs=10) as qp,
            tc.tile_pool(name="sqp", bufs=3) as sqp,
            tc.tile_pool(name="accp", bufs=4) as accp,
            tc.tile_pool(name="psA", bufs=NG, space="PSUM") as psA,
        ):
            # ---- replicated parameters ----
            bin_sb = consts.tile([1, M], F16)
            nc.scalar.dma_start(bin_sb[:, :], binit[:, :])
            onesx = consts.tile([1, 128], F16)
            nc.scalar.dma_start(onesx[:, :], onesr[:, :])
            negh = consts.tile([128, 1], F16)
            nc.scalar.dma_start(negh[:, :], neghc[:, :])
            vw_sb = consts.tile([128, KTILES * M], F16)
            nc.gpsimd.dma_start(vw_sb[:, :], vwi[:, :])
            y_sb = consts.tile([128, NG * NBLK], F32)

            # Dummy sigmoid: loads the sigmoid act table (which also contains
            # Square) once, early, so no later activation pays the 1283ns
            # table load.
            dum = consts.tile([1, 1], F32)
            nc.scalar.activation(dum[:, :], bin_sb[0:1, 0:1], AF.Sigmoid)

            # ACT-issued x quads, in ACT's FIFO before its epilogue work
            xts = {}
            for r in ACT_EARLY:
                xt = xin.tile([128, NQ * GCOLS], F16, name=f"x{r}", tag="x")
                nc.scalar.dma_start(xt[:, :], xq[r * 128:(r + 1) * 128, :])
                xts[r] = xt

            ps = [psA.tile([128, NBLK * M], F32, name=f"ps{g}", tag="ps")
                  for g in range(NG)]

            # init: zero psum and add b0 into col 64 of each block
            for g in range(NG):
                for blk in range(NBLK):
                    nc.tensor.matmul(
                        ps[g][:, blk * M:(blk + 1) * M],
                        onesx[0:1, :], bin_sb[0:1, :],
                        start=True, stop=False, skip_group_check=True,
                    )

            # ---- phase A: all remaining x DMAs + A-stream matmuls ----
            for r in range(KTILES):
                g, j = r // NQ, r % NQ
                if r in xts:
                    xt = xts[r]
                else:
                    xt = xin.tile([128, NQ * GCOLS], F16, name=f"x{r}", tag="x")
                    getattr(nc, DMA_ENG[r]).dma_start(
                        xt[:, :], xq[r * 128:(r + 1) * 128, :])
                    xts[r] = xt
                for t in range(NQ):
                    k = NQ * j + t
                    vw_k = vw_sb[:, k * M:(k + 1) * M]
                    for blk in range(NBLK):
                        sl = slice(t * GCOLS + blk * 128,
                                   t * GCOLS + (blk + 1) * 128)
                        nc.tensor.matmul(
                            ps[g][:, blk * M:(blk + 1) * M],
                            xt[:, sl], vw_k,
                            start=False, stop=j == NQ - 1 and t == NQ - 1,
                            skip_group_check=True,
                        )

            # ---- phase B: squares + B-stream matmuls, in SQB_ORDER ----
            b_last = {}   # group -> last quad in emission order
            for r in SQB_ORDER:
                b_last[r // NQ] = r
            for r in SQB_ORDER:
                g = r // NQ
                xt = xts[r]
                q = qp.tile([128, NQ * GCOLS], F16, name=f"q{r}", tag="q")
                se = SQ_ENG.get(r, "vector")
                if se == "scalar":
                    nc.scalar.activation(q[:, :], xt[:, :], AF.Square)
                else:
                    getattr(nc, se).tensor_mul(q[:, :], xt[:, :], xt[:, :])
                stop = b_last[g] == r
                for t in range(NQ):
                    for blk in range(NBLK):
                        sl = slice(t * GCOLS + blk * 128,
                                   t * GCOLS + (blk + 1) * 128)
                        nc.tensor.matmul(
                            ps[g][:, blk * M + EMBED:blk * M + M],
                            q[:, sl], negh[:, :],
                            start=False, stop=stop and t == NQ - 1,
                            skip_group_check=True,
                        )

            # ---- epilogue, per group ----
            for g in range(NG):
                for blk in range(NBLK):
                    sqo = sqp.tile([128, EMBED], F16, name=f"sq{g}_{blk}",
                                   tag="sq")
                    acc = accp.tile([128, 1], F32, name=f"acc{g}_{blk}",
                                    tag="acc")
                    nc.scalar.activation(
                        sqo[:, :], ps[g][:, blk * M:blk * M + EMBED],
                        AF.Square, scale=SQRT_HALF, accum_out=acc[:, :])
                    nc.scalar.activation(
                        y_sb[:, g * NBLK + blk:g * NBLK + blk + 1],
                        ps[g][:, blk * M + EMBED:blk * M + M],
                        AF.Sigmoid, bias=acc[:, :])

            # y writeback: emitted after all x DMAs so the waits on sigmoid
            # sems never block SP's x-DMA stream.
            for g in range(NG):
                nc.sync.dma_start(
                    y[:, g * NBLK:(g + 1) * NBLK],
                    y_sb[:, g * NBLK:(g + 1) * NBLK])

    nc.compile()
    return nc


_NC_CACHE = None


def _prep_inputs(x, w, b, v):
    x = np.asarray(x, dtype=np.float32)
    w = np.asarray(w, dtype=np.float32).reshape(FIELD, 1)
    v = np.asarray(v, dtype=np.float32)
    b0 = float(np.asarray(b, dtype=np.float32).reshape(-1)[0])

    s64 = (v.astype(np.float64) ** 2).sum(axis=1)
    sqs = np.sqrt(np.maximum(s64, 1e-38))
    vw = np.concatenate([v, w], axis=1).astype(np.float64)  # [FIELD, M]
    vwp = (vw / sqs[:, None]).astype(np.float16)

    # [128, KTILES*M] image: stripe k -> cols [k*M:(k+1)*M]
    vwi = np.ascontiguousarray(
        vwp.reshape(KTILES, 128, M).transpose(1, 0, 2).reshape(128, KTILES * M))

    binit = np.zeros((1, M), np.float16)
    binit[0, EMBED] = np.float16(b0)
    onesr = np.ones((1, 128), np.float16)
    neghc = np.full((128, 1), -0.5, np.float16)

    sqs16 = sqs.astype(np.float32)
    in_maps = []
    for c in range(NCORES):
        xs = x[c * BS:(c + 1) * BS, :]                     # [BS, FIELD]
        ut = (xs * sqs16[None, :]).T.astype(np.float16)    # [FIELD, BS]
        # xq[(g*4+j)*128 + p, t*GCOLS + cc] = ut[(4j+t)*128 + p, g*GCOLS + cc]
        u5 = ut.reshape(NQ, NQ, 128, NG, GCOLS)            # [j, t, p, g, cc]
        xqc = np.ascontiguousarray(
            u5.transpose(3, 0, 2, 1, 4).reshape(KTILES * 128, NQ * GCOLS))
        in_maps.append({"xq": xqc, "vwi": vwi, "binit": binit,
                        "onesr": onesr, "neghc": neghc})
    return in_maps


def _run(x, w, b, v, **spmd_kwargs):
    global _NC_CACHE
    if _NC_CACHE is None:
        _NC_CACHE = _build_nc()
    nc = _NC_CACHE

    in_maps = _prep_inputs(x, w, b, v)
    res = run_bass_kernel_spmd(nc, in_maps, list(range(NCORES)), **spmd_kwargs)
    # y[p, bl] holds batch element bl*128 + p of the core's shard
    out = np.concatenate(
        [res.results[c]["y"].T.reshape(BS) for c in range(NCORES)]
    )
    return out.reshape(BATCH, 1).astype(np.float32), res


def kernel(x, w, b, v):
    out, _ = _run(x, w, b, v)
    return out


# revision 20
# speedup vs baseline: 3.6430x; 1.1325x over previous
"""DeepFM forward kernel for 8 Trainium2 NeuronCores (Bass/Tile).

Math (per batch row b):
    lin[b] = x[b] @ w + b0
    C[b]   = sum_k (x[b] @ v)_k^2
    Bq[b]  = sum_f s[f] * x[b,f]^2,   s[f] = sum_k v[f,k]^2
    out[b] = sigmoid(lin[b] + 0.5*C[b] - 0.5*Bq[b])

Data-parallel: batch 16384 sharded 8 ways (2048 rows/core); parameters
replicated.

Scheme (fp16 data path, ~1e-3 output error, gate is 2e-2):
  - Host ships u = sqrt(s) * x, transposed (features on partitions), fp16,
    packed as 16 "quad" tiles: quad r=(g,j) holds stripes 4j..4j+3 of
    feature rows for batch-column group g.  vw' = [v | w] / sqrt(s) fp16.
  - A-stream: x-stationary matmuls: lhsT = u block [128f x 128b], rhs =
    vw'_k [128f x 65] -> psum block [128b x 65] accumulated over the 16
    feature stripes.  Column 64 is the linear term.
  - B-stream: q = u*u (elementwise, split across DVE/ACT/Pool), then
    lhsT = q block, rhs = [-0.5] -> accumulates -0.5*Bq INTO psum col 64.
  - b0 enters via one start=True matmul per block: lhsT = ones[1x128],
    rhs = binit[1x65] (col64 = b0), which also zero-initializes psum.
  - Epilogue per block: ACT Square(psum[:,0:64], scale=sqrt(0.5),
    accum_out=acc) then ACT Sigmoid(in=psum[:,64], bias=acc).
  - x DMAs alternate SP/ACT queues (transfers on different queues overlap
    in HW); squares are split DVE(12)/ACT(2)/Pool(2) to balance engines.
"""

import numpy as np

import concourse.bass as bass
import concourse.tile as tile
from concourse import bacc, mybir
from concourse.bass_utils import run_bass_kernel_spmd

BATCH, FIELD, EMBED = 16384, 2048, 64
NCORES = 8
BS = BATCH // NCORES       # 2048 batch rows per core
KTILES = FIELD // 128      # 16 feature stripes
NG = 4                     # batch-column groups per core
GCOLS = BS // NG           # 512 cols per group
NBLK = GCOLS // 128        # 4 batch blocks per group
NQ = 4                     # quads (of 4 stripes) per group
M = EMBED + 1              # 65: v columns plus w

F32 = mybir.dt.float32
F32R = mybir.dt.float32r
F16 = mybir.dt.float16
AF = mybir.ActivationFunctionType

SQRT_HALF = 0.7071067811865476

# Engine assignment knobs, indexed by quad r = g*NQ + j (16 quads).
# DMA queues: transfers on different queues overlap, but each DMA blocks its
# issuing engine for the whole transfer. SP (no compute) carries 8 quads,
# Pool 5 + the vw image, ACT 3 - issued EARLY in its FIFO while it would
# otherwise idle waiting for the epilogue dependencies.
DMA_ENG = {0: "sync", 2: "sync", 4: "sync", 6: "sync",
           8: "sync", 10: "sync", 12: "sync", 14: "sync",
           1: "gpsimd", 3: "gpsimd", 5: "gpsimd", 7: "gpsimd",
           9: "gpsimd", 11: "gpsimd",
           13: "scalar", 15: "scalar"}
ACT_EARLY = [13, 15]       # ACT-issued quads, emitted before the main loop
# squares: DVE fp16 2x mode is cheapest (1127ns/quad); Pool squares run
# after its DMA stream drains; ACT takes two once its own DMAs land.
SQ_ENG = {9: "gpsimd", 11: "gpsimd", 12: "gpsimd",
          15: "scalar", 7: "scalar"}  # rest DVE
# square+B emission order (a permutation of quads): all x DMAs are emitted
# first, so each engine's square FIFO follows this order. DVE's sub-order
# tracks data arrival; quad 14 (SP's last) is deliberately DVE-last so the
# queue is drained when it lands.
SQB_ORDER = [0, 1, 2, 3, 4, 5, 15, 13, 6, 7, 8, 9, 10, 11, 12, 14]
# epilogue: all ACT (DVE tensor_tensor_reduce can't read two PSUM inputs).
EPI_DVE_GROUPS = set()


def _build_nc():
    nc = bacc.Bacc("TRN2", target_bir_lowering=False, debug=False)

    # xq row-block r=g*4+j: [128, 4*GCOLS] image; cols [t*GCOLS:(t+1)*GCOLS]
    # hold stripe 4j+t restricted to group g's batch columns.
    xq = nc.declare_dram_parameter("xq", [KTILES * 128, NQ * GCOLS], F16,
                                   isOutput=False)
    vwi = nc.declare_dram_parameter("vwi", [128, KTILES * M], F16, isOutput=False)
    binit = nc.declare_dram_parameter("binit", [1, M], F16, isOutput=False)
    onesr = nc.declare_dram_parameter("onesr", [1, 128], F16, isOutput=False)
    neghc = nc.declare_dram_parameter("neghc", [128, 1], F16, isOutput=False)
    y = nc.declare_dram_parameter("y", [128, NG * NBLK], F32, isOutput=True)

    with tile.TileContext(nc) as tc:
        with (
            tc.tile_pool(name="consts", bufs=1) as consts,
            tc.tile_pool(name="xin", bufs=16) as xin,
            tc.tile_pool(name="qp", bufs=10) as qp,
            tc.tile_pool(name="sqp", bufs=3) as sqp,
            tc.tile_pool(name="accp", bufs=4) as accp,
            tc.tile_pool(name="psA", bufs=NG, space="PSUM") as psA,
        ):
            # ---- replicated parameters ----
            bin_sb = consts.tile([1, M], F16)
            nc.scalar.dma_start(bin_sb[:, :], binit[:, :])
            onesx = consts.tile([1, 128], F16)
            nc.scalar.dma_start(onesx[:, :], onesr[:, :])
            negh = consts.tile([128, 1], F16)
            nc.scalar.dma_start(negh[:, :], neghc[:, :])
            vw_sb = consts.tile([128, KTILES * M], F16)
            nc.gpsimd.dma_start(vw_sb[:, :], vwi[:, :])
            y_sb = consts.tile([128, NG * NBLK], F32)

            # Dummy sigmoid: loads the sigmoid act table (which also contains
            # Square) once, early, so no later activation pays the 1283ns
            # table load.
            dum = consts.tile([1, 1], F32)
            nc.scalar.activation(dum[:, :], bin_sb[0:1, 0:1], AF.Sigmoid)

            # ACT-issued x quads, in ACT's FIFO before its epilogue work
            xts = {}
            for r in ACT_EARLY:
                xt = xin.tile([128, NQ * GCOLS], F16, name=f"x{r}", tag="x")
                nc.scalar.dma_start(xt[:, :], xq[r * 128:(r + 1) * 128, :])
                xts[r] = xt

            ps = [psA.tile([128, NBLK * M], F32, name=f"ps{g}", tag="ps")
                  for g in range(NG)]

            # init: zero psum and add b0 into col 64 of each block
            for g in range(NG):
                for blk in range(NBLK):
                    nc.tensor.matmul(
                        ps[g][:, blk * M:(blk + 1) * M],
                        onesx[0:1, :], bin_sb[0:1, :],
                        start=True, stop=False, skip_group_check=True,
                    )

            # ---- phase A: all remaining x DMAs + A-stream matmuls ----
            for r in range(KTILES):
                g, j = r // NQ, r % NQ
                if r in xts:
                    xt = xts[r]
                else:
                    xt = xin.tile([128, NQ * GCOLS], F16, name=f"x{r}", tag="x")
                    getattr(nc, DMA_ENG[r]).dma_start(
                        xt[:, :], xq[r * 128:(r + 1) * 128, :])
                    xts[r] = xt
                for t in range(NQ):
                    k = NQ * j + t
                    vw_k = vw_sb[:, k * M:(k + 1) * M]
                    for blk in range(NBLK):
                        sl = slice(t * GCOLS + blk * 128,
                                   t * GCOLS + (blk + 1) * 128)
                        nc.tensor.matmul(
                            ps[g][:, blk * M:(blk + 1) * M],
                            xt[:, sl], vw_k,
                            start=False, stop=j == NQ - 1 and t == NQ - 1,
                            skip_group_check=True,
                        )

            # ---- phase B: squares + B-stream matmuls, in SQB_ORDER ----
            b_last = {}   # group -> last quad in emission order
            for r in SQB_ORDER:
                b_last[r // NQ] = r
            for r in SQB_ORDER:
                g = r // NQ
                xt = xts[r]
                q = qp.tile([128, NQ * GCOLS], F16, name=f"q{r}", tag="q")
                se = SQ_ENG.get(r, "vector")
                if se == "scalar":
                    nc.scalar.activation(q[:, :], xt[:, :], AF.Square)
                else:
                    getattr(nc, se).tensor_mul(q[:, :], xt[:, :], xt[:, :])
                stop = b_last[g] == r
                for t in range(NQ):
                    for blk in range(NBLK):
                        sl = slice(t * GCOLS + blk * 128,
                                   t * GCOLS + (blk + 1) * 128)
                        nc.tensor.matmul(
                            ps[g][:, blk * M + EMBED:blk * M + M],
                            q[:, sl], negh[:, :],
                            start=False, stop=stop and t == NQ - 1,
                            skip_group_check=True,
                        )

            # ---- epilogue, per group ----
            for g in range(NG):
                for blk in range(NBLK):
                    sqo = sqp.tile([128, EMBED], F16, name=f"sq{g}_{blk}",
                                   tag="sq")
                    acc = accp.tile([128, 1], F32, name=f"acc{g}_{blk}",
                                    tag="acc")
                    if g in EPI_DVE_GROUPS:
                        nc.vector.tensor_tensor_reduce(
                            sqo[:, :], ps[g][:, blk * M:blk * M + EMBED],
                            ps[g][:, blk * M:blk * M + EMBED], 0.5, 0.0,
                            mybir.AluOpType.mult, mybir.AluOpType.add,
                            acc[:, :])
                    else:
                        nc.scalar.activation(
                            sqo[:, :], ps[g][:, blk * M:blk * M + EMBED],
                            AF.Square, scale=SQRT_HALF, accum_out=acc[:, :])
                    nc.scalar.activation(
                        y_sb[:, g * NBLK + blk:g * NBLK + blk + 1],
                        ps[g][:, blk * M + EMBED:blk * M + M],
                        AF.Sigmoid, bias=acc[:, :])

            # y writeback: emitted after all x DMAs so the waits on sigmoid
            # sems never block SP's x-DMA stream.
            for g in range(NG):
                nc.sync.dma_start(
                    y[:, g * NBLK:(g + 1) * NBLK],
                    y_sb[:, g * NBLK:(g + 1) * NBLK])

    nc.compile()
    return nc


_NC_CACHE = None


def _prep_inputs(x, w, b, v):
    x = np.asarray(x, dtype=np.float32)
    w = np.asarray(w, dtype=np.float32).reshape(FIELD, 1)
    v = np.asarray(v, dtype=np.float32)
    b0 = float(np.asarray(b, dtype=np.float32).reshape(-1)[0])

    s64 = (v.astype(np.float64) ** 2).sum(axis=1)
    sqs = np.sqrt(np.maximum(s64, 1e-38))
    vw = np.concatenate([v, w], axis=1).astype(np.float64)  # [FIELD, M]
    vwp = (vw / sqs[:, None]).astype(np.float16)

    # [128, KTILES*M] image: stripe k -> cols [k*M:(k+1)*M]
    vwi = np.ascontiguousarray(
        vwp.reshape(KTILES, 128, M).transpose(1, 0, 2).reshape(128, KTILES * M))

    binit = np.zeros((1, M), np.float16)
    binit[0, EMBED] = np.float16(b0)
    onesr = np.ones((1, 128), np.float16)
    neghc = np.full((128, 1), -0.5, np.float16)

    sqs16 = sqs.astype(np.float32)
    in_maps = []
    for c in range(NCORES):
        xs = x[c * BS:(c + 1) * BS, :]                     # [BS, FIELD]
        ut = (xs * sqs16[None, :]).T.astype(np.float16)    # [FIELD, BS]
        # xq[(g*4+j)*128 + p, t*GCOLS + cc] = ut[(4j+t)*128 + p, g*GCOLS + cc]
        u5 = ut.reshape(NQ, NQ, 128, NG, GCOLS)            # [j, t, p, g, cc]
        xqc = np.ascontiguousarray(
            u5.transpose(3, 0, 2, 1, 4).reshape(KTILES * 128, NQ * GCOLS))
        in_maps.append({"xq": xqc, "vwi": vwi, "binit": binit,
                        "onesr": onesr, "neghc": neghc})
    return in_maps


def _run(x, w, b, v, **spmd_kwargs):
    global _NC_CACHE
    if _NC_CACHE is None:
        _NC_CACHE = _build_nc()
    nc = _NC_CACHE

    in_maps = _prep_inputs(x, w, b, v)
    res = run_bass_kernel_spmd(nc, in_maps, list(range(NCORES)), **spmd_kwargs)
    # y[p, bl] holds batch element bl*128 + p of the core's shard
    out = np.concatenate(
        [res.results[c]["y"].T.reshape(BS) for c in range(NCORES)]
    )
    return out.reshape(BATCH, 1).astype(np.float32), res


def kernel(x, w, b, v):
    out, _ = _run(x, w, b, v)
    return out


# revision 26
# speedup vs baseline: 3.8152x; 1.0473x over previous
"""DeepFM forward kernel for 8 Trainium2 NeuronCores (Bass/Tile).

Math (per batch row b):
    lin[b] = x[b] @ w + b0
    C[b]   = sum_k (x[b] @ v)_k^2
    Bq[b]  = sum_f s[f] * x[b,f]^2,   s[f] = sum_k v[f,k]^2
    out[b] = sigmoid(lin[b] + 0.5*C[b] - 0.5*Bq[b])

Data-parallel: batch 16384 sharded 8 ways (2048 rows/core); parameters
replicated.

Scheme (fp16 data path, ~6e-4 output error, gate is 2e-2):
  - Host ships u = sqrt(s) * x, transposed (features on partitions), fp16,
    packed as 16 "quad" tiles: quad r=(g,j) holds stripes 4j..4j+3 of
    feature rows for batch-column group g.  vw' = [v | w] / sqrt(s) fp16.
  - A-stream, x-stationary: lhsT = u block [128f x 128b], rhs = v'_k
    [128f x 64] -> psxv[g] block [128b x 64], plus rhs = w'_k [128f x 1]
    -> pslin[g] col. Accumulated over the 16 feature stripes.
  - B-stream: q = u*u (elementwise, split DVE/ACT/Pool), then lhsT = q
    block, rhs = [-0.5] accumulates -0.5*Bq into the same pslin col.
    b0 enters via one start=True matmul per group into pslin.
  - Epilogue per group: ONE ACT Square over psxv [128,256] (scale
    sqrt(0.5)) -> sqo fp16, ONE DVE tensor_reduce [128,4,64]->[128,4],
    then 4 ACT Sigmoids (in=pslin col, bias=acc col).
  - x DMAs: SP 8 quads, Pool 6 + vw image, ACT 2 (issued early in its
    FIFO). Transfers on different queues overlap; each DMA blocks its
    issuing engine for the transfer duration.
"""

import numpy as np

import concourse.bass as bass
import concourse.tile as tile
from concourse import bacc, mybir
from concourse.bass_utils import run_bass_kernel_spmd

BATCH, FIELD, EMBED = 16384, 2048, 64
NCORES = 8
BS = BATCH // NCORES       # 2048 batch rows per core
KTILES = FIELD // 128      # 16 feature stripes
NG = 4                     # batch-column groups per core
GCOLS = BS // NG           # 512 cols per group
NBLK = GCOLS // 128        # 4 batch blocks per group
NQ = 4                     # quads (of 4 stripes) per group
M = EMBED + 1              # 65: v columns plus w

F32 = mybir.dt.float32
F16 = mybir.dt.float16
AF = mybir.ActivationFunctionType
ALU = mybir.AluOpType
AX = mybir.AxisListType

SQRT_HALF = 0.7071067811865476

# Engine assignment knobs, indexed by quad r = g*NQ + j (16 quads).
DMA_ENG = {0: "sync", 2: "sync", 4: "sync", 6: "sync",
           8: "sync", 10: "sync", 12: "sync", 14: "sync",
           1: "gpsimd", 3: "gpsimd", 5: "gpsimd", 7: "gpsimd",
           9: "gpsimd", 11: "gpsimd",
           13: "scalar", 15: "scalar"}
ACT_EARLY = [13, 15]       # ACT-issued quads, emitted before the main loop
SQ_ENG = {9: "gpsimd", 11: "gpsimd", 12: "gpsimd",
          15: "scalar", 7: "scalar"}  # rest DVE
# square+B emission order (a permutation of quads). DVE's sub-order tracks
# data arrival; quad 14 (SP's last) is DVE-last so the queue is drained
# when it lands.
SQB_ORDER = [0, 1, 2, 3, 4, 5, 15, 13, 6, 7, 8, 9, 10, 11, 12, 14]


def _build_nc():
    nc = bacc.Bacc("TRN2", target_bir_lowering=False, debug=False)

    xq = nc.declare_dram_parameter("xq", [KTILES * 128, NQ * GCOLS], F16,
                                   isOutput=False)
    vwi = nc.declare_dram_parameter("vwi", [128, KTILES * M], F16, isOutput=False)
    binit = nc.declare_dram_parameter("binit", [1, NBLK], F16, isOutput=False)
    zrow = nc.declare_dram_parameter("zrow", [1, NBLK * EMBED], F16,
                                     isOutput=False)
    onesr = nc.declare_dram_parameter("onesr", [1, 128], F16, isOutput=False)
    neghc = nc.declare_dram_parameter("neghc", [128, 1], F16, isOutput=False)
    y = nc.declare_dram_parameter("y", [128, NG * NBLK], F32, isOutput=True)

    with tile.TileContext(nc) as tc:
        with (
            tc.tile_pool(name="consts", bufs=1) as consts,
            tc.tile_pool(name="xin", bufs=16) as xin,
            tc.tile_pool(name="qp", bufs=10) as qp,
            tc.tile_pool(name="sqp", bufs=4) as sqp,
            tc.tile_pool(name="accp", bufs=4) as accp,
            tc.tile_pool(name="psx", bufs=NG, space="PSUM") as psx,
            tc.tile_pool(name="psl", bufs=NG, space="PSUM") as psl,
        ):
            # ---- replicated parameters ----
            bin_sb = consts.tile([1, NBLK], F16)
            nc.scalar.dma_start(bin_sb[:, :], binit[:, :])
            z_sb = consts.tile([1, NBLK * EMBED], F16)
            nc.scalar.dma_start(z_sb[:, :], zrow[:, :])
            onesx = consts.tile([1, 128], F16)
            nc.scalar.dma_start(onesx[:, :], onesr[:, :])
            negh = consts.tile([128, 1], F16)
            nc.scalar.dma_start(negh[:, :], neghc[:, :])
            vw_sb = consts.tile([128, KTILES * M], F16)
            nc.gpsimd.dma_start(vw_sb[:, :], vwi[:, :])
            y_sb = consts.tile([128, NG * NBLK], F32)

            # table pre-load: the first activation charges its set load while
            # the pipeline is still waiting on x data.
            dum = consts.tile([1, 1], F32)
            nc.scalar.activation(dum[:, :], bin_sb[0:1, 0:1], AF.Sigmoid)

            # ACT-issued x quads, in ACT's FIFO before its epilogue work
            xts = {}
            for r in ACT_EARLY:
                xt = xin.tile([128, NQ * GCOLS], F16, name=f"x{r}", tag="x")
                nc.scalar.dma_start(xt[:, :], xq[r * 128:(r + 1) * 128, :])
                xts[r] = xt

            psxv = [psx.tile([128, NBLK, EMBED], F32, name=f"psx{g}", tag="px")
                    for g in range(NG)]
            pslin = [psl.tile([128, NBLK], F32, name=f"psl{g}", tag="pl")
                     for g in range(NG)]

            # init: start=True zeroes the whole PSUM *bank*, so emit all
            # bank-zeroing matmuls first (any cascade overwrites only zeros),
            # then add b0 with start=False.
            for g in range(NG):
                nc.tensor.matmul(
                    psxv[g][:, :, :], onesx[0:1, :], z_sb[0:1, :],
                    start=True, stop=False, skip_group_check=True,
                )
                nc.tensor.matmul(
                    pslin[g][:, :], onesx[0:1, :], z_sb[0:1, 0:NBLK],
                    start=True, stop=False, skip_group_check=True,
                )
            for g in range(NG):
                nc.tensor.matmul(
                    pslin[g][:, :], onesx[0:1, :], bin_sb[0:1, :],
                    start=False, stop=False, skip_group_check=True,
                )

            # ---- phase A: x DMAs + A-stream matmuls ----
            for r in range(KTILES):
                g, j = r // NQ, r % NQ
                if r in xts:
                    xt = xts[r]
                else:
                    xt = xin.tile([128, NQ * GCOLS], F16, name=f"x{r}", tag="x")
                    getattr(nc, DMA_ENG[r]).dma_start(
                        xt[:, :], xq[r * 128:(r + 1) * 128, :])
                    xts[r] = xt
                first, last = j == 0, j == NQ - 1
                for t in range(NQ):
                    k = NQ * j + t
                    v_k = vw_sb[:, k * M:k * M + EMBED]
                    w_k = vw_sb[:, k * M + EMBED:(k + 1) * M]
                    for blk in range(NBLK):
                        sl = slice(t * GCOLS + blk * 128,
                                   t * GCOLS + (blk + 1) * 128)
                        nc.tensor.matmul(
                            psxv[g][:, blk, :],
                            xt[:, sl], v_k,
                            start=False, stop=last and t == NQ - 1,
                            skip_group_check=True,
                        )
                        nc.tensor.matmul(
                            pslin[g][:, blk:blk + 1],
                            xt[:, sl], w_k,
                            start=False, stop=False,
                            skip_group_check=True,
                        )

            # ---- phase B: squares + B matmuls, epilogue woven in ----
            b_last = {}
            for i, r in enumerate(SQB_ORDER):
                b_last[r // NQ] = i

            for i, r in enumerate(SQB_ORDER):
                g = r // NQ
                xt = xts[r]
                q = qp.tile([128, NQ * GCOLS], F16, name=f"q{r}", tag="q")
                se = SQ_ENG.get(r, "vector")
                if se == "scalar":
                    nc.scalar.activation(q[:, :], xt[:, :], AF.Square)
                else:
                    getattr(nc, se).tensor_mul(q[:, :], xt[:, :], xt[:, :])
                stop = b_last[g] == i
                for t in range(NQ):
                    for blk in range(NBLK):
                        sl = slice(t * GCOLS + blk * 128,
                                   t * GCOLS + (blk + 1) * 128)
                        nc.tensor.matmul(
                            pslin[g][:, blk:blk + 1],
                            q[:, sl], negh[:, :],
                            start=False, stop=stop and t == NQ - 1,
                            skip_group_check=True,
                        )
                if stop:
                    # ---- group epilogue ----
                    sqo = sqp.tile([128, NBLK, EMBED], F16, name=f"sq{g}",
                                   tag="sq")
                    nc.scalar.activation(
                        sqo[:, :, :], psxv[g][:, :, :],
                        AF.Square, scale=SQRT_HALF)
                    acc = accp.tile([128, NBLK], F32, name=f"acc{g}",
                                    tag="acc")
                    nc.vector.tensor_reduce(
                        acc[:, :], sqo[:, :, :], AX.X, ALU.add)
                    for blk in range(NBLK):
                        nc.scalar.activation(
                            y_sb[:, g * NBLK + blk:g * NBLK + blk + 1],
                            pslin[g][:, blk:blk + 1],
                            AF.Sigmoid, bias=acc[:, blk:blk + 1])

            # y writeback: emitted last so the sigmoid waits never block
            # SP's x-DMA stream.
            for g in range(NG):
                nc.sync.dma_start(
                    y[:, g * NBLK:(g + 1) * NBLK],
                    y_sb[:, g * NBLK:(g + 1) * NBLK])

    nc.compile()
    return nc


_NC_CACHE = None


def _prep_inputs(x, w, b, v):
    x = np.asarray(x, dtype=np.float32)
    w = np.asarray(w, dtype=np.float32).reshape(FIELD, 1)
    v = np.asarray(v, dtype=np.float32)
    b0 = float(np.asarray(b, dtype=np.float32).reshape(-1)[0])

    s64 = (v.astype(np.float64) ** 2).sum(axis=1)
    sqs = np.sqrt(np.maximum(s64, 1e-38))
    vw = np.concatenate([v, w], axis=1).astype(np.float64)  # [FIELD, M]
    vwp = (vw / sqs[:, None]).astype(np.float16)

    vwi = np.ascontiguousarray(
        vwp.reshape(KTILES, 128, M).transpose(1, 0, 2).reshape(128, KTILES * M))

    binit = np.full((1, NBLK), b0, np.float16)
    zrow = np.zeros((1, NBLK * EMBED), np.float16)
    onesr = np.ones((1, 128), np.float16)
    neghc = np.full((128, 1), -0.5, np.float16)

    sqs32 = sqs.astype(np.float32)
    in_maps = []
    for c in range(NCORES):
        xs = x[c * BS:(c + 1) * BS, :]                     # [BS, FIELD]
        ut = (xs * sqs32[None, :]).T.astype(np.float16)    # [FIELD, BS]
        # xq[(g*4+j)*128 + p, t*GCOLS + cc] = ut[(4j+t)*128 + p, g*GCOLS + cc]
        u5 = ut.reshape(NQ, NQ, 128, NG, GCOLS)            # [j, t, p, g, cc]
        xqc = np.ascontiguousarray(
            u5.transpose(3, 0, 2, 1, 4).reshape(KTILES * 128, NQ * GCOLS))
        in_maps.append({"xq": xqc, "vwi": vwi, "binit": binit, "zrow": zrow,
                        "onesr": onesr, "neghc": neghc})
    return in_maps


def _run(x, w, b, v, **spmd_kwargs):
    global _NC_CACHE
    if _NC_CACHE is None:
        _NC_CACHE = _build_nc()
    nc = _NC_CACHE

    in_maps = _prep_inputs(x, w, b, v)
    res = run_bass_kernel_spmd(nc, in_maps, list(range(NCORES)), **spmd_kwargs)
    # y[p, bl] holds batch element bl*128 + p of the core's shard
    out = np.concatenate(
        [res.results[c]["y"].T.reshape(BS) for c in range(NCORES)]
    )
    return out.reshape(BATCH, 1).astype(np.float32), res


def kernel(x, w, b, v):
    out, _ = _run(x, w, b, v)
    return out
